# revision 23
# baseline (speedup 1.0000x reference)
"""Weighted-L1 loss kernel for Trainium2 (8 NeuronCores, data-parallel).

Computes: mean_i( sum_j w[j] * |inputs[i,j] - targets[i,j]| )
for inputs/targets [16384, 4096] f32, w [4096] f32.

Strategy: shard rows across 8 cores (2048 rows each). Per core, for each
[128, 4096] row-tile:
    VectorE: d = a - b            (f32 -> bf16)
    ScalarE: e = |d|              (activation Abs, bf16)
    TensorE: colsum += ones.T @ e (contract partition dim, accumulate PSUM f32)
Device output per core: colsum[j] = sum_i |a[i,j]-b[i,j]|  -- a [1, 4096] f32.
Host: loss = (sum_over_cores(colsum) . w) / B.  (w >= 0 is not needed on
device at all since the weighting is a per-column linear postscale.)
"""

import os
import numpy as np

try:
    import concourse.bass as bass
except ImportError:  # pragma: no cover
    import sys

    sys.path.insert(0, "/opt/trn_rl_repo")
    import concourse.bass as bass

import concourse.bacc as bacc
import concourse.mybir as mybir
import concourse.tile as tile
from concourse.bass_utils import run_bass_kernel_spmd

B, D = 16384, 4096
NCORES = 8
R = B // NCORES  # 2048 rows per core
P = 128  # SBUF partitions
NT = R // P  # 16 row-tiles per core
MM_N = 512  # PSUM bank width in f32
NJ = D // MM_N  # 8 column chunks

_NC_CACHE = {}


def _build_nc():
    nc = bacc.Bacc("TRN2", target_bir_lowering=False, debug=False)
    a = nc.dram_tensor("a", [R, D], mybir.dt.float32, kind="ExternalInput")
    b = nc.dram_tensor("b", [R, D], mybir.dt.float32, kind="ExternalInput")
    out = nc.dram_tensor("colsum", [1, D], mybir.dt.float32, kind="ExternalOutput")

    CK = 512  # tail chunk width (1 PSUM bank)
    NCK = D // CK

    with tile.TileContext(nc) as tc:
        with (
            tc.tile_pool(name="ioa", bufs=5) as ioa_pool,
            tc.tile_pool(name="iob", bufs=4) as iob_pool,
            tc.tile_pool(name="bl", bufs=4) as bl_pool,
            tc.tile_pool(name="d", bufs=2) as d_pool,
            tc.tile_pool(name="e", bufs=2) as e_pool,
            tc.tile_pool(name="const", bufs=1) as const_pool,
            tc.tile_pool(name="acc", bufs=1, space=bass.MemorySpace.PSUM) as psum_pool,
        ):
            ones = const_pool.tile([P, 1], mybir.dt.bfloat16)
            nc.gpsimd.memset(ones[:], 1.0)

            acc = psum_pool.tile([1, D], mybir.dt.float32)

            def absdiff_mm(at_ap, bt_ap, width, col0, start, stop):
                d = d_pool.tile([P, width], mybir.dt.bfloat16, tag="d")
                nc.vector.tensor_sub(d[:], at_ap, bt_ap)
                e = e_pool.tile([P, width], mybir.dt.bfloat16, tag="e")
                nc.vector.tensor_scalar(
                    e[:].bitcast(mybir.dt.uint16),
                    d[:].bitcast(mybir.dt.uint16),
                    0x7FFF,
                    None,
                    op0=mybir.AluOpType.bitwise_and,
                )
                for jt in range(width // MM_N):
                    c = col0 + jt * MM_N
                    nc.tensor.matmul(
                        acc[:, c : c + MM_N],
                        ones[:],
                        e[:, jt * MM_N : (jt + 1) * MM_N],
                        start=start,
                        stop=stop,
                    )

            for it in range(NT - 1):
                at = ioa_pool.tile([P, D], mybir.dt.float32, tag="a")
                bt = iob_pool.tile([P, D], mybir.dt.float32, tag="b")
                nc.sync.dma_start(at[:], a[it * P : (it + 1) * P, :])
                nc.scalar.dma_start(bt[:], b[it * P : (it + 1) * P, :])
                absdiff_mm(at[:], bt[:], D, 0, it == 0, False)

            # Last row-tile: chunk the b-load and pipeline the tail so only
            # one small chunk's compute remains after the final byte lands.
            # All chunk DMA issues are consecutive on the scalar sequencer;
            # PSUM copies come only after every stop-matmul (no compute ever
            # sits between DMA issues in an engine's program).
            it = NT - 1
            at = ioa_pool.tile([P, D], mybir.dt.float32, tag="a")
            nc.sync.dma_start(at[:], a[it * P : (it + 1) * P, :])
            btcs = []
            for ck in range(NCK):
                cs = slice(ck * CK, (ck + 1) * CK)
                btc = bl_pool.tile([P, CK], mybir.dt.float32, tag="bl")
                nc.scalar.dma_start(btc[:], b[it * P : (it + 1) * P, cs])
                btcs.append(btc)
            for ck in range(NCK):
                cs = slice(ck * CK, (ck + 1) * CK)
                absdiff_mm(at[:, cs], btcs[ck][:], CK, ck * CK, False, True)
            for ck in range(NCK):
                cs = slice(ck * CK, (ck + 1) * CK)
                res = ioa_pool.tile([1, CK], mybir.dt.float32, tag="a")
                nc.scalar.copy(res[:], acc[:, cs])
                nc.sync.dma_start(out[:, cs], res[:])

    nc.compile()
    return nc


def run(inputs, targets, w, trace=False, **spmd_kwargs):
    """Run the sharded kernel; returns (loss_scalar, BassKernelResults)."""
    key = "nc"
    if key not in _NC_CACHE:
        _NC_CACHE[key] = _build_nc()
    nc = _NC_CACHE[key]

    inputs = np.asarray(inputs, dtype=np.float32)
    targets = np.asarray(targets, dtype=np.float32)
    w = np.asarray(w, dtype=np.float32)

    in_maps = [
        {
            "a": inputs[c * R : (c + 1) * R],
            "b": targets[c * R : (c + 1) * R],
        }
        for c in range(NCORES)
    ]
    res = run_bass_kernel_spmd(
        nc, in_maps, list(range(NCORES)), trace=trace, **spmd_kwargs
    )
    total = np.zeros(D, dtype=np.float64)
    for c in range(NCORES):
        total += res.results[c]["colsum"][0].astype(np.float64)
    loss = (total * w.astype(np.float64)).sum() / B
    return np.asarray(loss, dtype=np.float32), res


def kernel(inputs, targets, w):
    loss, _ = run(inputs, targets, w, trace=False)
    return loss
